# revision 6
# baseline (speedup 1.0000x reference)
"""Trainium2 Bass kernel for nn_DMMRLoss (siamese 3D-CNN patch loss).

Pipeline (per core, 16 streams = 4 row-groups x 4 col-groups, S=18):
  - host: bbox, extract 17^3 patches (13^3=2197 -> pad 2304), keep-mask
  - host: partial im2col for conv1 (z,y gathered, x whole), 18 rows/patch, fp8
  - device:
      conv1: per 2-patch group, per dx: ONE [128,128] ldweights + 32
        non-self-loading matmuls (16 streams x 2 patches), K=18, N=343
      evac: relu+bias PSUM->SBUF bf16, one [128, 4x343] copy per patch
        (ACT and DVE alternate patches)
      conv2: single pass, all 8 psum banks hold 16 streams' [27*18] outputs;
        per offset o: ONE [128,128] ldweights + 48 matmuls (oz'-split, N=162)
      cc evac: relu+bias, pos-major layout so fc1 rhs is contiguous
      fc1: per (pos,h): ONE [128,128] ldweights + 2 matmuls (v row-halves)
      fc2 (bf16, no tanh) -> [1, 288] fp32 out
  - host: +fc2_bias, tanh, weighted mean over kept patches

All matmuls after warmup use ldweights=False with explicit nc.tensor.ldweights
of [128,128] stationaries (weights host-duplicated across PE subtiles), so
the weight-load port is ~25x less loaded than one-LDW-per-matmul.
"""
import sys

sys.path.insert(0, '/opt/trn_rl_repo')

import numpy as np
import ml_dtypes

import concourse.bacc as bacc
import concourse.mybir as mybir
import concourse.tile as tile
from concourse import bass_utils
from concourse.ap import AP


PATCH = 17
THRESH = 0.5
NCORES = 8
NR = 4            # row groups (stream dim 1)
NCG = 4           # col groups (stream dim 2)
S = 18            # patches per stream
NG = 3            # X DMA groups
JG = S // NG      # patches per DMA group
AF = mybir.ActivationFunctionType

DT = mybir.dt.bfloat16
NPDT = ml_dtypes.bfloat16
DT8 = mybir.dt.float8e4
NPDT8 = ml_dtypes.float8_e4m3

PF = 735              # per-row X elems: oz7*oy7*x15
C1P = 343             # conv1 out positions per patch (7^3)
FC1 = NR * S * C1P    # c1 free size: 24696
FX = NCG * S * PF     # x free size: 52920
NOUT = 2 * 8 * S      # 288 outputs per core


def _ap(a, dims, off=0):
    return AP(tensor=a.tensor, offset=a.offset + off, ap=[list(d) for d in dims])


_cache = {}


def _build():
    if 'nc' in _cache:
        return _cache['nc']

    nc = bacc.Bacc("TRN2", target_bir_lowering=False, debug=False,
                   num_devices=NCORES)

    x_d = nc.dram_tensor("x", (NG, NR, 18, NCG * JG * PF), DT8,
                         kind="ExternalInput")
    w1_d = nc.dram_tensor("w1", (128, 3 * 128), DT8, kind="ExternalInput")
    w2_d = nc.dram_tensor("w2", (128, 27 * 128), DT, kind="ExternalInput")
    wf1_d = nc.dram_tensor("wf1", (128, 54 * 128), DT, kind="ExternalInput")
    wf2_d = nc.dram_tensor("wf2", (128, 2), DT, kind="ExternalInput")
    b1_d = nc.dram_tensor("b1", (128, 1), mybir.dt.float32, kind="ExternalInput")
    b2_d = nc.dram_tensor("b2", (128, 1), mybir.dt.float32, kind="ExternalInput")
    bf1_d = nc.dram_tensor("bf1", (128, 2), mybir.dt.float32, kind="ExternalInput")
    o_d = nc.dram_tensor("o", (1, NOUT), mybir.dt.float32, kind="ExternalOutput")

    def mm(out, lhsT, rhs, start, stop, tp):
        m = nc.tensor.matmul(out, lhsT, rhs, start=start, stop=stop,
                             tile_position=tp)
        m.ldweights = False
        return m

    with tile.TileContext(nc) as tc:
        with (
            tc.tile_pool(name="const", bufs=1) as cpool,
            tc.tile_pool(name="xin", bufs=1) as xpool,
            tc.tile_pool(name="c1", bufs=1) as c1pool,
            tc.tile_pool(name="cc", bufs=1) as ccpool,
            tc.tile_pool(name="fin", bufs=1) as fpool,
            tc.tile_pool(name="ps", bufs=2, space="PSUM") as pspool,
        ):
            w1 = cpool.tile([128, 3 * 128], DT8)
            dum = cpool.tile([32, 512], DT8)
            nc.vector.memset(dum[:], 0.0)
            w2 = cpool.tile([128, 27 * 128], DT)
            wf1 = cpool.tile([128, 54 * 128], DT)
            wf2 = cpool.tile([128, 2], DT)
            b1 = cpool.tile([128, 1], mybir.dt.float32)
            b2 = cpool.tile([128, 1], mybir.dt.float32)
            bf1 = cpool.tile([128, 2], mybir.dt.float32)

            x = xpool.tile([128, FX], DT8)
            c1 = c1pool.tile([128, FC1], DT)
            cc = ccpool.tile([128, 27 * 8 * S], DT)   # [pos27][sl8][j18]
            f1 = fpool.tile([128, 2 * NOUT], DT)
            out_sb = fpool.tile([1, NOUT], mybir.dt.float32)

            # --- DMAs: weights on gpsimd, X split across sync+gpsimd ---
            nc.gpsimd.dma_start(w1[:], w1_d[:])
            nc.sync.dma_start(b1[:], b1_d[:])
            nc.sync.dma_start(b2[:], b2_d[:])
            nc.sync.dma_start(bf1[:], bf1_d[:])
            nc.sync.dma_start(wf2[:], wf2_d[:])
            for g in range(NG):
                for r in range(NR):
                    eng = nc.sync if r < 2 else nc.gpsimd
                    dst = _ap(x[:], [[FX, 18], [S * PF, NCG], [1, JG * PF]],
                              off=32 * r * FX + g * JG * PF)
                    src = _ap(x_d[g, r],
                              [[NCG * JG * PF, 18], [JG * PF, NCG],
                               [1, JG * PF]])
                    eng.dma_start(dst, src)
            nc.gpsimd.dma_start(w2[:], w2_d[:])
            nc.gpsimd.dma_start(wf1[:], wf1_d[:])

            # --- PE warmup (HAM un-throttle) during X DMA wait ---
            warm = pspool.tile([128, 2048], mybir.dt.float32, tag="ps",
                               name="warm")
            for _ in range(16):
                nc.tensor.matmul(warm[0:32, 0:343], w1[0:32, 0:32],
                                 dum[0:32, 0:343], start=True, stop=True,
                                 tile_position=(0, 0))

            xr = x[:].rearrange("p (c j oz oy xx) -> p c j oz oy xx",
                                c=NCG, j=S, oz=7, oy=7)

            # --- conv1: 9 groups of 2 patch-indices ---
            for grp in range(S // 2):
                pa = pspool.tile([128, 2048], mybir.dt.float32, tag="ps",
                                 name=f"pa{grp}")
                pb = pspool.tile([128, 2048], mybir.dt.float32, tag="ps",
                                 name=f"pb{grp}")
                pq = (pa, pb)
                for dx in range(3):
                    nc.tensor.ldweights(w1[:, dx * 128:(dx + 1) * 128])
                    for q in range(2):
                        j = grp * 2 + q
                        for c in range(NCG):
                            for r in range(NR):
                                mm(pq[q][32 * c:32 * c + 32,
                                         r * 512:r * 512 + 343],
                                   w1[32 * r:32 * r + 18,
                                      dx * 128 + 32 * c:dx * 128 + 32 * c + 32],
                                   xr[32 * r:32 * r + 18, c, j, :, :,
                                      dx:dx + 13:2],
                                   start=(dx == 0), stop=(dx == 2),
                                   tp=(32 * r, 32 * c))
                # evac: relu+bias, one copy per patch; alternate engines
                for q in range(2):
                    j = grp * 2 + q
                    src = _ap(pq[q][:], [[2048, 128], [512, 4], [1, 343]])
                    dst = _ap(c1[:], [[FC1, 128], [S * C1P, 4], [1, C1P]],
                              off=j * C1P)
                    if (grp + q) % 2 == 0:
                        nc.scalar.activation(dst, src, AF.Relu,
                                             bias=b1[:, 0:1])
                    else:
                        nc.vector.tensor_scalar(
                            dst, src, b1[:, 0:1], 0.0,
                            op0=mybir.AluOpType.add, op1=mybir.AluOpType.max)

            # --- conv2: single pass, 16 streams resident in 8 banks ---
            # stream (r,c): v=r%2 (psum partition half), sl=(r//2)*4+c (bank)
            p2a = pspool.tile([128, 2048], mybir.dt.float32, tag="ps",
                              name="p2a")
            p2b = pspool.tile([128, 2048], mybir.dt.float32, tag="ps",
                              name="p2b")
            for o in range(27):
                dz, dy, dx = o // 9, (o // 3) % 3, o % 3
                nc.tensor.ldweights(w2[:, o * 128:(o + 1) * 128])
                for c in range(NCG):
                    for r in range(NR):
                        v = r % 2
                        sl = (r // 2) * 4 + c
                        pt = p2a if sl < 4 else p2b
                        bo = (sl % 4) * 512
                        for ozp in range(3):
                            rhs = _ap(c1[:],
                                      [[FC1, 32], [C1P, S], [14, 3], [2, 3]],
                                      off=32 * c * FC1 + r * S * C1P
                                          + (2 * ozp + dz) * 49 + dy * 7 + dx)
                            mm(pt[64 * v:64 * v + 64,
                                  bo + ozp * 162:bo + ozp * 162 + 162],
                               w2[32 * c:32 * c + 32,
                                  o * 128 + 64 * v:o * 128 + 64 * v + 64],
                               rhs, start=(o == 0 and ozp == 0),
                               stop=(o == 26),
                               tp=(32 * c, 64 * v))
            # cc evac: [j][pos] -> [pos][sl][j], relu+bias
            for half, pt in enumerate((p2a, p2b)):
                for i in range(4):
                    sl = half * 4 + i
                    src = _ap(pt[:], [[2048, 128], [162, 3], [1, 9], [9, S]],
                              off=i * 512)
                    dst = _ap(cc[:],
                              [[27 * 8 * S, 128], [9 * 8 * S, 3], [8 * S, 9],
                               [1, S]],
                              off=sl * S)
                    if i % 2 == 0:
                        nc.scalar.activation(dst, src, AF.Relu,
                                             bias=b2[:, 0:1])
                    else:
                        nc.vector.tensor_scalar(
                            dst, src, b2[:, 0:1], 0.0,
                            op0=mybir.AluOpType.add, op1=mybir.AluOpType.max)

            # --- fc1: 27 pos x 2 oc-halves; one [128,128] LDW each ---
            psf0 = pspool.tile([128, 2048], mybir.dt.float32, tag="ps",
                               name="psf0")
            psf1 = pspool.tile([128, 2048], mybir.dt.float32, tag="ps",
                               name="psf1")
            psf = (psf0, psf1)
            for pos in range(27):
                for h in range(2):
                    ch = pos * 2 + h
                    nc.tensor.ldweights(wf1[:, ch * 128:(ch + 1) * 128])
                    for v in range(2):
                        mm(psf[h][0:128, v * 512:v * 512 + 144],
                           wf1[64 * v:64 * v + 64, ch * 128:(ch + 1) * 128],
                           cc[64 * v:64 * v + 64, pos * 144:pos * 144 + 144],
                           start=(pos == 0),
                           stop=(pos == 26), tp=(64 * v, 0))
            srcf0 = _ap(psf0[:], [[2048, 128], [512, 2], [1, 144]])
            dstf0 = _ap(f1[:], [[2 * NOUT, 128], [144, 2], [1, 144]])
            nc.scalar.activation(dstf0, srcf0, AF.Relu, bias=bf1[:, 0:1])
            srcf1 = _ap(psf1[:], [[2048, 128], [512, 2], [1, 144]])
            dstf1 = _ap(f1[:], [[2 * NOUT, 128], [144, 2], [1, 144]], off=NOUT)
            nc.vector.tensor_scalar(dstf1, srcf1, bf1[:, 1:2], 0.0,
                                    op0=mybir.AluOpType.add,
                                    op1=mybir.AluOpType.max)

            # --- fc2 (bf16, self-loading; no tanh - host does it) ---
            psf2 = pspool.tile([128, 2048], mybir.dt.float32, tag="ps",
                               name="psf2")
            for h in range(2):
                nc.tensor.matmul(psf2[0:1, 0:NOUT], wf2[:, h:h + 1],
                                 f1[:, h * NOUT:(h + 1) * NOUT],
                                 start=(h == 0), stop=(h == 1),
                                 tile_position=(0, 0))
            nc.scalar.copy(out_sb[:], psf2[0:1, 0:NOUT])
            nc.sync.dma_start(o_d[:], out_sb[:])

    nc.compile()
    _cache['nc'] = nc
    return nc


def _bbox(mask):
    zs = np.flatnonzero(mask.any(axis=(1, 2)))
    ys = np.flatnonzero(mask.any(axis=(0, 2)))
    xs = np.flatnonzero(mask.any(axis=(0, 1)))
    return (int(xs[0]), int(ys[0]), int(zs[0]),
            int(xs[-1]), int(ys[-1]), int(zs[-1]))


def _extract(vol, bbox):
    x0, y0, z0, x1, y1, z1 = bbox
    t = vol[0, 0, z0:z1, y0:y1, x0:x1]
    pads = []
    for d in t.shape:
        rr = d % PATCH
        p = (PATCH - rr) % PATCH
        pads.append((p // 2, p - p // 2))
    t = np.pad(t, pads)
    D, H, W = t.shape
    nD, nH, nW = D // PATCH, H // PATCH, W // PATCH
    p = t.reshape(nD, PATCH, nH, PATCH, nW, PATCH)
    return p.transpose(0, 2, 4, 1, 3, 5).reshape(-1, PATCH, PATCH, PATCH)


def kernel(source, target, conv1_w, conv1_b, conv2_w, conv2_b,
           fc1_w, fc1_b, fc2_w, fc2_b):
    source = np.asarray(source, np.float32)
    target = np.asarray(target, np.float32)
    conv1_w = np.asarray(conv1_w, np.float32)
    conv1_b = np.asarray(conv1_b, np.float32)
    conv2_w = np.asarray(conv2_w, np.float32)
    conv2_b = np.asarray(conv2_b, np.float32)
    fc1_w = np.asarray(fc1_w, np.float32)
    fc1_b = np.asarray(fc1_b, np.float32)
    fc2_w = np.asarray(fc2_w, np.float32)
    fc2_b = np.asarray(fc2_b, np.float32)

    bbox = _bbox(target[0, 0] > 0)
    fixed = _extract(target, bbox)
    moving = _extract(source, bbox)
    Np = fixed.shape[0]
    keep = ((fixed == 0).reshape(Np, -1).mean(axis=1) <= THRESH).astype(np.float32)

    Npad = NCORES * NR * NCG * S   # 2304
    assert Np <= Npad, (Np, Npad)

    nc = _build()

    # --- X: partial im2col [18 rows=(ci,dz,dy)] x [735=(oz7,oy7,x15)] ---
    P2 = np.zeros((Npad, 2, PATCH, PATCH, PATCH), np.float32)
    P2[:Np, 0] = fixed
    P2[:Np, 1] = moving
    s0, s1, s2, s3, s4 = P2.strides
    cols = np.lib.stride_tricks.as_strided(
        P2, (Npad, 2, 3, 3, 7, 7, 15),
        (s0, s1, s2, s3, 2 * s2, 2 * s3, s4))
    # patch p = ((core*4 + r)*4 + c)*S + j ; device wants [g][r][row18][c][jj]
    colsr = cols.reshape(NCORES, NR, NCG, NG, JG, 18, PF)
    X8 = np.ascontiguousarray(
        colsr.transpose(0, 3, 1, 5, 2, 4, 6)).astype(NPDT8)
    # X8: [core][g][r][row18][c][jj][735]

    # --- weights (all duplicated to [128,128] stationaries) ---
    w1t = conv1_w.transpose(1, 2, 3, 4, 0).reshape(18, 3, 32)  # (ci,dz,dy),dx,co
    W1 = np.zeros((4, 32, 3, 4, 32), np.float32)   # [r][row][dx][c][co]
    W1[:, :18] = w1t[None, :, :, None, :]
    W1 = W1.reshape(128, 3 * 128).astype(NPDT8)  # [32r+k, dx*128+32c+co]

    w2t = conv2_w.transpose(1, 2, 3, 4, 0).reshape(32, 27, 64)  # ci,o,co
    W2 = np.zeros((4, 32, 27, 2, 64), np.float32)  # [c][ci][o][v][co]
    W2[:] = w2t[None, :, :, None, :]
    W2 = W2.reshape(128, 27 * 128).astype(NPDT)

    wf1t = fc1_w.reshape(2, 128, 64, 27)           # [h][oc][co][pos]
    A = wf1t.transpose(2, 3, 0, 1).reshape(64, 54 * 128)  # [co][(pos,h)*128+oc]
    WF1 = np.concatenate([A, A], axis=0).astype(NPDT)     # [128=(v,co), 6912]

    WF2 = fc2_w.reshape(2, 128).T.copy().astype(NPDT)     # [128, 2] (col h)
    B1 = np.tile(conv1_b, 4).reshape(128, 1).astype(np.float32)
    B2 = np.tile(conv2_b, 2).reshape(128, 1).astype(np.float32)
    BF1 = fc1_b.reshape(2, 128).T.copy().astype(np.float32)

    in_maps = []
    for core in range(NCORES):
        in_maps.append({
            "x": np.ascontiguousarray(X8[core]).reshape(NG, NR, 18,
                                                        NCG * JG * PF),
            "w1": W1, "w2": W2, "wf1": WF1, "wf2": WF2,
            "b1": B1, "b2": B2, "bf1": BF1,
        })

    res = bass_utils.run_bass_kernel_spmd(nc, in_maps,
                                          core_ids=list(range(NCORES)))
    global _last_results
    _last_results = res

    # --- gather: out col = v*144 + sl*18 + j ; r=(sl//4)*2+v, c=sl%4 ---
    y = np.zeros(Npad, np.float32)
    o = np.stack([res.results[core]["o"][0] for core in range(NCORES)])
    ov = o.reshape(NCORES, 2, 8, S)                # core, v, sl, j
    for v in range(2):
        for sl in range(8):
            r = (sl // 4) * 2 + v
            c = sl % 4
            base = (r * NCG + c) * S
            for core in range(NCORES):
                y[core * NR * NCG * S + base:
                  core * NR * NCG * S + base + S] = ov[core, v, sl]

    yt = np.tanh(y + fc2_b[0])
    out = np.sum(yt[:Np] * keep) / np.sum(keep)
    return np.float32(out)


# revision 7
# speedup vs baseline: 1.0005x; 1.0005x over previous
"""Trainium2 Bass kernel for nn_DMMRLoss (siamese 3D-CNN patch loss).

Pipeline (per core, 16 streams = 4 row-groups x 4 col-groups, S=18):
  - host: bbox, extract 17^3 patches (13^3=2197 -> pad 2304), keep-mask
  - host: partial im2col for conv1 (z,y gathered, x whole), 18 rows/patch, fp8
  - device:
      conv1: per 2-patch group, per dx: ONE [128,128] ldweights + 32
        non-self-loading matmuls (16 streams x 2 patches), K=18, N=343
      evac: relu+bias PSUM->SBUF bf16, one [128, 4x343] copy per patch
        (ACT and DVE alternate patches)
      conv2: single pass, all 8 psum banks hold 16 streams' [27*18] outputs;
        per offset o: ONE [128,128] ldweights + 48 matmuls (oz'-split, N=162)
      cc evac: relu+bias, pos-major layout so fc1 rhs is contiguous
      fc1: per (pos,h): ONE [128,128] ldweights + 2 matmuls (v row-halves)
      fc2 (bf16, no tanh) -> [1, 288] fp32 out
  - host: +fc2_bias, tanh, weighted mean over kept patches

All matmuls after warmup use ldweights=False with explicit nc.tensor.ldweights
of [128,128] stationaries (weights host-duplicated across PE subtiles), so
the weight-load port is ~25x less loaded than one-LDW-per-matmul.
"""
import sys

sys.path.insert(0, '/opt/trn_rl_repo')

import numpy as np
import ml_dtypes

import concourse.bacc as bacc
import concourse.mybir as mybir
import concourse.tile as tile
from concourse import bass_utils
from concourse.ap import AP


PATCH = 17
THRESH = 0.5
NCORES = 8
NR = 4            # row groups (stream dim 1)
NCG = 4           # col groups (stream dim 2)
S = 18            # patches per stream
NG = 3            # X DMA groups
JG = S // NG      # patches per DMA group
AF = mybir.ActivationFunctionType

DT = mybir.dt.bfloat16
NPDT = ml_dtypes.bfloat16
DT8 = mybir.dt.float8e4
NPDT8 = ml_dtypes.float8_e4m3

PF = 735              # per-row X elems: oz7*oy7*x15
C1P = 343             # conv1 out positions per patch (7^3)
FC1 = NR * S * C1P    # c1 free size: 24696
FX = NCG * S * PF     # x free size: 52920
NOUT = 2 * 8 * S      # 288 outputs per core


def _ap(a, dims, off=0):
    return AP(tensor=a.tensor, offset=a.offset + off, ap=[list(d) for d in dims])


_cache = {}


def _build():
    if 'nc' in _cache:
        return _cache['nc']

    nc = bacc.Bacc("TRN2", target_bir_lowering=False, debug=False,
                   num_devices=NCORES)

    x_d = nc.dram_tensor("x", (NG, NR, 18, NCG * JG * PF), DT8,
                         kind="ExternalInput")
    w1_d = nc.dram_tensor("w1", (128, 3 * 128), DT8, kind="ExternalInput")
    w2_d = nc.dram_tensor("w2", (128, 27 * 128), DT, kind="ExternalInput")
    wf1_d = nc.dram_tensor("wf1", (128, 54 * 128), DT, kind="ExternalInput")
    wf2_d = nc.dram_tensor("wf2", (128, 2), DT, kind="ExternalInput")
    b1_d = nc.dram_tensor("b1", (128, 1), mybir.dt.float32, kind="ExternalInput")
    b2_d = nc.dram_tensor("b2", (128, 1), mybir.dt.float32, kind="ExternalInput")
    bf1_d = nc.dram_tensor("bf1", (128, 2), mybir.dt.float32, kind="ExternalInput")
    o_d = nc.dram_tensor("o", (1, NOUT), mybir.dt.float32, kind="ExternalOutput")

    def mm(out, lhsT, rhs, start, stop, tp):
        m = nc.tensor.matmul(out, lhsT, rhs, start=start, stop=stop,
                             tile_position=tp)
        m.ins.ldweights = False
        return m

    with tile.TileContext(nc) as tc:
        with (
            tc.tile_pool(name="const", bufs=1) as cpool,
            tc.tile_pool(name="xin", bufs=1) as xpool,
            tc.tile_pool(name="c1", bufs=1) as c1pool,
            tc.tile_pool(name="cc", bufs=1) as ccpool,
            tc.tile_pool(name="fin", bufs=1) as fpool,
            tc.tile_pool(name="ps", bufs=2, space="PSUM") as pspool,
        ):
            w1 = cpool.tile([128, 3 * 128], DT8)
            dum = cpool.tile([32, 512], DT8)
            nc.vector.memset(dum[:], 0.0)
            w2 = cpool.tile([128, 27 * 128], DT)
            wf1 = cpool.tile([128, 54 * 128], DT)
            wf2 = cpool.tile([128, 2], DT)
            b1 = cpool.tile([128, 1], mybir.dt.float32)
            b2 = cpool.tile([128, 1], mybir.dt.float32)
            bf1 = cpool.tile([128, 2], mybir.dt.float32)

            x = xpool.tile([128, FX], DT8)
            c1 = c1pool.tile([128, FC1], DT)
            cc = ccpool.tile([128, 27 * 8 * S], DT)   # [pos27][sl8][j18]
            f1 = fpool.tile([128, 2 * NOUT], DT)
            out_sb = fpool.tile([1, NOUT], mybir.dt.float32)

            # --- DMAs: weights on gpsimd, X split across sync+gpsimd ---
            nc.gpsimd.dma_start(w1[:], w1_d[:])
            nc.sync.dma_start(b1[:], b1_d[:])
            nc.sync.dma_start(b2[:], b2_d[:])
            nc.sync.dma_start(bf1[:], bf1_d[:])
            nc.sync.dma_start(wf2[:], wf2_d[:])
            for g in range(NG):
                for r in range(NR):
                    eng = nc.sync if r < 2 else nc.gpsimd
                    dst = _ap(x[:], [[FX, 18], [S * PF, NCG], [1, JG * PF]],
                              off=32 * r * FX + g * JG * PF)
                    src = _ap(x_d[g, r],
                              [[NCG * JG * PF, 18], [JG * PF, NCG],
                               [1, JG * PF]])
                    eng.dma_start(dst, src)
            nc.gpsimd.dma_start(w2[:], w2_d[:])
            nc.gpsimd.dma_start(wf1[:], wf1_d[:])

            # --- PE warmup (HAM un-throttle) during X DMA wait ---
            warm = pspool.tile([128, 2048], mybir.dt.float32, tag="ps",
                               name="warm")
            for _ in range(16):
                nc.tensor.matmul(warm[0:32, 0:343], w1[0:32, 0:32],
                                 dum[0:32, 0:343], start=True, stop=True,
                                 tile_position=(0, 0))

            xr = x[:].rearrange("p (c j oz oy xx) -> p c j oz oy xx",
                                c=NCG, j=S, oz=7, oy=7)

            # --- conv1: 9 groups of 2 patch-indices ---
            for grp in range(S // 2):
                pa = pspool.tile([128, 2048], mybir.dt.float32, tag="ps",
                                 name=f"pa{grp}")
                pb = pspool.tile([128, 2048], mybir.dt.float32, tag="ps",
                                 name=f"pb{grp}")
                pq = (pa, pb)
                for dx in range(3):
                    nc.tensor.ldweights(w1[:, dx * 128:(dx + 1) * 128])
                    for q in range(2):
                        j = grp * 2 + q
                        for c in range(NCG):
                            for r in range(NR):
                                mm(pq[q][32 * c:32 * c + 32,
                                         r * 512:r * 512 + 343],
                                   w1[32 * r:32 * r + 18,
                                      dx * 128 + 32 * c:dx * 128 + 32 * c + 32],
                                   xr[32 * r:32 * r + 18, c, j, :, :,
                                      dx:dx + 13:2],
                                   start=(dx == 0), stop=(dx == 2),
                                   tp=(32 * r, 32 * c))
                # evac: relu+bias, one copy per patch; alternate engines
                for q in range(2):
                    j = grp * 2 + q
                    src = _ap(pq[q][:], [[2048, 128], [512, 4], [1, 343]])
                    dst = _ap(c1[:], [[FC1, 128], [S * C1P, 4], [1, C1P]],
                              off=j * C1P)
                    if (grp + q) % 2 == 0:
                        nc.scalar.activation(dst, src, AF.Relu,
                                             bias=b1[:, 0:1])
                    else:
                        nc.vector.tensor_scalar(
                            dst, src, b1[:, 0:1], 0.0,
                            op0=mybir.AluOpType.add, op1=mybir.AluOpType.max)

            # --- conv2: single pass, 16 streams resident in 8 banks ---
            # stream (r,c): v=r%2 (psum partition half), sl=(r//2)*4+c (bank)
            p2a = pspool.tile([128, 2048], mybir.dt.float32, tag="ps",
                              name="p2a")
            p2b = pspool.tile([128, 2048], mybir.dt.float32, tag="ps",
                              name="p2b")
            for o in range(27):
                dz, dy, dx = o // 9, (o // 3) % 3, o % 3
                nc.tensor.ldweights(w2[:, o * 128:(o + 1) * 128])
                for c in range(NCG):
                    for r in range(NR):
                        v = r % 2
                        sl = (r // 2) * 4 + c
                        pt = p2a if sl < 4 else p2b
                        bo = (sl % 4) * 512
                        for ozp in range(3):
                            rhs = _ap(c1[:],
                                      [[FC1, 32], [C1P, S], [14, 3], [2, 3]],
                                      off=32 * c * FC1 + r * S * C1P
                                          + (2 * ozp + dz) * 49 + dy * 7 + dx)
                            mm(pt[64 * v:64 * v + 64,
                                  bo + ozp * 162:bo + ozp * 162 + 162],
                               w2[32 * c:32 * c + 32,
                                  o * 128 + 64 * v:o * 128 + 64 * v + 64],
                               rhs, start=(o == 0 and ozp == 0),
                               stop=(o == 26),
                               tp=(32 * c, 64 * v))
            # cc evac: [j][pos] -> [pos][sl][j], relu+bias
            for half, pt in enumerate((p2a, p2b)):
                for i in range(4):
                    sl = half * 4 + i
                    src = _ap(pt[:], [[2048, 128], [162, 3], [1, 9], [9, S]],
                              off=i * 512)
                    dst = _ap(cc[:],
                              [[27 * 8 * S, 128], [9 * 8 * S, 3], [8 * S, 9],
                               [1, S]],
                              off=sl * S)
                    if i % 2 == 0:
                        nc.scalar.activation(dst, src, AF.Relu,
                                             bias=b2[:, 0:1])
                    else:
                        nc.vector.tensor_scalar(
                            dst, src, b2[:, 0:1], 0.0,
                            op0=mybir.AluOpType.add, op1=mybir.AluOpType.max)

            # --- fc1: 27 pos x 2 oc-halves; one [128,128] LDW each ---
            psf0 = pspool.tile([128, 2048], mybir.dt.float32, tag="ps",
                               name="psf0")
            psf1 = pspool.tile([128, 2048], mybir.dt.float32, tag="ps",
                               name="psf1")
            psf = (psf0, psf1)
            for pos in range(27):
                for h in range(2):
                    ch = pos * 2 + h
                    nc.tensor.ldweights(wf1[:, ch * 128:(ch + 1) * 128])
                    for v in range(2):
                        mm(psf[h][0:128, v * 512:v * 512 + 144],
                           wf1[64 * v:64 * v + 64, ch * 128:(ch + 1) * 128],
                           cc[64 * v:64 * v + 64, pos * 144:pos * 144 + 144],
                           start=(pos == 0),
                           stop=(pos == 26), tp=(64 * v, 0))
            srcf0 = _ap(psf0[:], [[2048, 128], [512, 2], [1, 144]])
            dstf0 = _ap(f1[:], [[2 * NOUT, 128], [144, 2], [1, 144]])
            nc.scalar.activation(dstf0, srcf0, AF.Relu, bias=bf1[:, 0:1])
            srcf1 = _ap(psf1[:], [[2048, 128], [512, 2], [1, 144]])
            dstf1 = _ap(f1[:], [[2 * NOUT, 128], [144, 2], [1, 144]], off=NOUT)
            nc.vector.tensor_scalar(dstf1, srcf1, bf1[:, 1:2], 0.0,
                                    op0=mybir.AluOpType.add,
                                    op1=mybir.AluOpType.max)

            # --- fc2 (bf16, self-loading; no tanh - host does it) ---
            psf2 = pspool.tile([128, 2048], mybir.dt.float32, tag="ps",
                               name="psf2")
            for h in range(2):
                nc.tensor.matmul(psf2[0:1, 0:NOUT], wf2[:, h:h + 1],
                                 f1[:, h * NOUT:(h + 1) * NOUT],
                                 start=(h == 0), stop=(h == 1),
                                 tile_position=(0, 0))
            nc.scalar.copy(out_sb[:], psf2[0:1, 0:NOUT])
            nc.sync.dma_start(o_d[:], out_sb[:])

    nc.compile()
    _cache['nc'] = nc
    return nc


def _bbox(mask):
    zs = np.flatnonzero(mask.any(axis=(1, 2)))
    ys = np.flatnonzero(mask.any(axis=(0, 2)))
    xs = np.flatnonzero(mask.any(axis=(0, 1)))
    return (int(xs[0]), int(ys[0]), int(zs[0]),
            int(xs[-1]), int(ys[-1]), int(zs[-1]))


def _extract(vol, bbox):
    x0, y0, z0, x1, y1, z1 = bbox
    t = vol[0, 0, z0:z1, y0:y1, x0:x1]
    pads = []
    for d in t.shape:
        rr = d % PATCH
        p = (PATCH - rr) % PATCH
        pads.append((p // 2, p - p // 2))
    t = np.pad(t, pads)
    D, H, W = t.shape
    nD, nH, nW = D // PATCH, H // PATCH, W // PATCH
    p = t.reshape(nD, PATCH, nH, PATCH, nW, PATCH)
    return p.transpose(0, 2, 4, 1, 3, 5).reshape(-1, PATCH, PATCH, PATCH)


def kernel(source, target, conv1_w, conv1_b, conv2_w, conv2_b,
           fc1_w, fc1_b, fc2_w, fc2_b):
    source = np.asarray(source, np.float32)
    target = np.asarray(target, np.float32)
    conv1_w = np.asarray(conv1_w, np.float32)
    conv1_b = np.asarray(conv1_b, np.float32)
    conv2_w = np.asarray(conv2_w, np.float32)
    conv2_b = np.asarray(conv2_b, np.float32)
    fc1_w = np.asarray(fc1_w, np.float32)
    fc1_b = np.asarray(fc1_b, np.float32)
    fc2_w = np.asarray(fc2_w, np.float32)
    fc2_b = np.asarray(fc2_b, np.float32)

    bbox = _bbox(target[0, 0] > 0)
    fixed = _extract(target, bbox)
    moving = _extract(source, bbox)
    Np = fixed.shape[0]
    keep = ((fixed == 0).reshape(Np, -1).mean(axis=1) <= THRESH).astype(np.float32)

    Npad = NCORES * NR * NCG * S   # 2304
    assert Np <= Npad, (Np, Npad)

    nc = _build()

    # --- X: partial im2col [18 rows=(ci,dz,dy)] x [735=(oz7,oy7,x15)] ---
    P2 = np.zeros((Npad, 2, PATCH, PATCH, PATCH), np.float32)
    P2[:Np, 0] = fixed
    P2[:Np, 1] = moving
    s0, s1, s2, s3, s4 = P2.strides
    cols = np.lib.stride_tricks.as_strided(
        P2, (Npad, 2, 3, 3, 7, 7, 15),
        (s0, s1, s2, s3, 2 * s2, 2 * s3, s4))
    # patch p = ((core*4 + r)*4 + c)*S + j ; device wants [g][r][row18][c][jj]
    colsr = cols.reshape(NCORES, NR, NCG, NG, JG, 18, PF)
    X8 = np.ascontiguousarray(
        colsr.transpose(0, 3, 1, 5, 2, 4, 6)).astype(NPDT8)
    # X8: [core][g][r][row18][c][jj][735]

    # --- weights (all duplicated to [128,128] stationaries) ---
    w1t = conv1_w.transpose(1, 2, 3, 4, 0).reshape(18, 3, 32)  # (ci,dz,dy),dx,co
    W1 = np.zeros((4, 32, 3, 4, 32), np.float32)   # [r][row][dx][c][co]
    W1[:, :18] = w1t[None, :, :, None, :]
    W1 = W1.reshape(128, 3 * 128).astype(NPDT8)  # [32r+k, dx*128+32c+co]

    w2t = conv2_w.transpose(1, 2, 3, 4, 0).reshape(32, 27, 64)  # ci,o,co
    W2 = np.zeros((4, 32, 27, 2, 64), np.float32)  # [c][ci][o][v][co]
    W2[:] = w2t[None, :, :, None, :]
    W2 = W2.reshape(128, 27 * 128).astype(NPDT)

    wf1t = fc1_w.reshape(2, 128, 64, 27)           # [h][oc][co][pos]
    A = wf1t.transpose(2, 3, 0, 1).reshape(64, 54 * 128)  # [co][(pos,h)*128+oc]
    WF1 = np.concatenate([A, A], axis=0).astype(NPDT)     # [128=(v,co), 6912]

    WF2 = fc2_w.reshape(2, 128).T.copy().astype(NPDT)     # [128, 2] (col h)
    B1 = np.tile(conv1_b, 4).reshape(128, 1).astype(np.float32)
    B2 = np.tile(conv2_b, 2).reshape(128, 1).astype(np.float32)
    BF1 = fc1_b.reshape(2, 128).T.copy().astype(np.float32)

    in_maps = []
    for core in range(NCORES):
        in_maps.append({
            "x": np.ascontiguousarray(X8[core]).reshape(NG, NR, 18,
                                                        NCG * JG * PF),
            "w1": W1, "w2": W2, "wf1": WF1, "wf2": WF2,
            "b1": B1, "b2": B2, "bf1": BF1,
        })

    res = bass_utils.run_bass_kernel_spmd(nc, in_maps,
                                          core_ids=list(range(NCORES)))
    global _last_results
    _last_results = res

    # --- gather: out col = v*144 + sl*18 + j ; r=(sl//4)*2+v, c=sl%4 ---
    y = np.zeros(Npad, np.float32)
    o = np.stack([res.results[core]["o"][0] for core in range(NCORES)])
    ov = o.reshape(NCORES, 2, 8, S)                # core, v, sl, j
    for v in range(2):
        for sl in range(8):
            r = (sl // 4) * 2 + v
            c = sl % 4
            base = (r * NCG + c) * S
            for core in range(NCORES):
                y[core * NR * NCG * S + base:
                  core * NR * NCG * S + base + S] = ov[core, v, sl]

    yt = np.tanh(y + fc2_b[0])
    out = np.sum(yt[:Np] * keep) / np.sum(keep)
    return np.float32(out)


# revision 10
# speedup vs baseline: 1.3591x; 1.3584x over previous
"""Trainium2 Bass kernel for nn_DMMRLoss (siamese 3D-CNN patch loss).

Pipeline:
  - host: bbox from target>0 mask, extract 17^3 patches (13^3=2197), keep-mask
  - host: partial im2col for conv1 (z,y gathered, x whole) + ones row for bias
  - device (8 NeuronCores, data-parallel over patches):
      conv1 (stride-2 3^3, 2->32ch): 3 accumulating matmuls (dx offsets),
        K=19 (=2ci*3dz*3dy + bias row), only the 7x7x7 positions conv2 reads
      relu+cast copies also z-gather (dz,oz') so conv2 rhs APs are 3-dim
      conv2 (stride-2 3^3, 32->64ch): 27 accumulating matmuls, strided APs
      fc1 (1728->256): 27 accumulating matmuls over (co,pos); relu
      fc2 (256->1) in fp32 + tanh
  - host: weighted mean over kept patches

Per core: 16 streams = 4 PE row-bases x 4 col-groups, S patches per stream.
X is packed 19 rows per row-base (no zero padding), one contiguous DMA per
(block, row-base). All PSUM tiles share one 8-bank ring; conv1 runs j-pairs
(8 banks = 4 row-bases x 2 patches) so weights stay loaded for 2 matmuls.
"""
import sys

sys.path.insert(0, '/opt/trn_rl_repo')

import numpy as np
import ml_dtypes

import concourse.bacc as bacc
import concourse.mybir as mybir
import concourse.tile as tile
from concourse import bass_utils
from concourse.ap import AP


PATCH = 17
THRESH = 0.5
NCORES = 8
NR = 4   # row bases (32-partition groups) for conv1
NCG = 4  # col groups (psum partition slices) for conv1
AF = mybir.ActivationFunctionType

DT = mybir.dt.bfloat16
NPDT = ml_dtypes.bfloat16
DT8 = mybir.dt.float8e4
NPDT8 = ml_dtypes.float8_e4m3

PF = 735            # per-patch per-row X elems: oz7*oy7*x15 (x>14 never read)
C1P = 147           # per (dz,j): oz'3 * y7 * x7
C1J = 3 * C1P       # per j incl dz: 441


def _ap(a, dims, off=0):
    r = AP(tensor=a.tensor, offset=a.offset + off, ap=[list(d) for d in dims])
    return r


def _pick_jb(S):
    for jb in (6, 2, 4, 3, 5, 1):
        if S % jb == 0:
            return jb
    return 1


_cache = {}


def _build(S, Jb):
    key = (S, Jb)
    if key in _cache:
        return _cache[key]
    NB = S // Jb
    FX = NCG * Jb * PF          # X tile free size per row-base
    FC1 = NR * S * C1J          # C1 tile free size (all S patches resident)

    nc = bacc.Bacc("TRN2", target_bir_lowering=False, debug=False,
                   num_devices=NCORES)

    x_d = nc.dram_tensor("x", (NB, 128, FX), DT8, kind="ExternalInput")
    w1_d = nc.dram_tensor("w1", (128, 3 * 32), DT8, kind="ExternalInput")
    w2_d = nc.dram_tensor("w2", (128, 27 * 64), DT, kind="ExternalInput")
    wf1_d = nc.dram_tensor("wf1", (128, 27 * 256), DT, kind="ExternalInput")
    wf2_d = nc.dram_tensor("wf2", (128, 2), mybir.dt.float32, kind="ExternalInput")
    b2_d = nc.dram_tensor("b2", (128, 1), mybir.dt.float32, kind="ExternalInput")
    bf1_d = nc.dram_tensor("bf1", (128, 2), mybir.dt.float32, kind="ExternalInput")
    bf2_d = nc.dram_tensor("bf2", (1, 1), mybir.dt.float32, kind="ExternalInput")
    o_d = nc.dram_tensor("o", (1, 2 * 8 * S), mybir.dt.float32,
                         kind="ExternalOutput")

    with tile.TileContext(nc) as tc:
        with (
            tc.tile_pool(name="const", bufs=1) as cpool,
            tc.tile_pool(name="xin", bufs=2) as xpool,
            tc.tile_pool(name="c1", bufs=1) as c1pool,
            tc.tile_pool(name="cc", bufs=1) as ccpool,
            tc.tile_pool(name="fin", bufs=1) as fpool,
            tc.tile_pool(name="ps", bufs=8, space="PSUM") as pspool,
        ):
            w1 = cpool.tile([128, 3 * 32], DT8)
            dum = cpool.tile([32, 512], DT8)
            nc.vector.memset(dum[:], 0.0)
            w2 = cpool.tile([128, 27 * 64], DT)
            wf1 = cpool.tile([128, 27 * 256], DT)
            wf2 = cpool.tile([128, 2], mybir.dt.float32)
            b2 = cpool.tile([128, 1], mybir.dt.float32)
            bf1 = cpool.tile([128, 2], mybir.dt.float32)
            bf2 = cpool.tile([1, 1], mybir.dt.float32)
            nc.sync.dma_start(w1[:], w1_d[:])
            nc.gpsimd.dma_start(w2[:], w2_d[:])
            nc.gpsimd.dma_start(wf1[:], wf1_d[:])
            nc.gpsimd.dma_start(wf2[:], wf2_d[:])
            nc.gpsimd.dma_start(b2[:], b2_d[:])
            nc.gpsimd.dma_start(bf1[:], bf1_d[:])
            nc.gpsimd.dma_start(bf2[:], bf2_d[:])

            # conv2 output staging for fc: [128=(v,co64), (slot8, S, 27)]
            cc = ccpool.tile([128, 8 * S * 27], DT)
            ccr = cc[:].rearrange("p (s j q) -> p s j q", s=8, j=S)

            # C1 layout per partition: (r, dz, j(all S), oz'3, y7, x7)
            c1 = c1pool.tile([128, FC1], DT)

            # PE warmup: ~6us of dummy matmuls so HAM un-throttles before
            # the first real conv1 matmul (they run during the X DMA wait)
            warm = pspool.tile([128, 343], mybir.dt.float32, tag="ps",
                               name="warm")
            for _ in range(24):
                nc.tensor.matmul(warm[0:32, :], w1[0:32, 0:32],
                                 dum[0:32, 0:343], start=True, stop=True,
                                 tile_position=(0, 0))

            for b in range(NB):
                x = xpool.tile([128, FX], DT8)
                xj = x[:].rearrange("p (c j f) -> p c j f", c=NCG, j=Jb)
                xdj = x_d[b].rearrange("p (c j f) -> p c j f", c=NCG, j=Jb)
                for g in range(0, Jb, 2):
                    eng = nc.sync if (g // 2) % 2 == 0 else nc.gpsimd
                    eng.dma_start(xj[:, :, g:g + 2, :], xdj[:, :, g:g + 2, :])
                xr = x[:].rearrange("p (c j oz oy xx) -> p c j oz oy xx",
                                    c=NCG, j=Jb, oz=7, oy=7)

                for j0 in range(0, Jb, 2):
                    jj = (j0, j0 + 1) if j0 + 1 < Jb else (j0,)
                    # 8 psum tiles (row-base r x patch jj); dx-outer, r-inner
                    # so weights stay loaded for len(jj) matmuls per tile and
                    # consecutive MMs hit different row groups
                    pss = {(q, rr): pspool.tile([128, 343], mybir.dt.float32,
                                                tag="ps",
                                                name=f"ps1_{b}_{j0}_{q}_{rr}")
                           for q in jj for rr in range(NR)}
                    for dx in range(3):
                        for q in jj:
                            for c in range(NCG):
                                for r in range(NR):
                                    nc.tensor.matmul(
                                        pss[(q, r)][32 * c:32 * c + 32, :],
                                        w1[32 * r:32 * r + 19,
                                           dx * 32:(dx + 1) * 32],
                                        xr[32 * r:32 * r + 19, c, q, :, :,
                                           dx:dx + 13:2],
                                        start=(dx == 0), stop=(dx == 2),
                                        tile_position=(32 * r, 32 * c),
                                    )
                    for q in jj:
                        jg = b * Jb + q
                        for r in range(NR):
                            # relu + z-gather + cast (bias folded into matmul)
                            src = _ap(pss[(q, r)][:],
                                      [[343, 128], [49, 3], [98, 3], [1, 49]])
                            dst = _ap(c1[:],
                                      [[FC1, 128], [S * C1P, 3], [1, C1P]],
                                      off=r * S * C1J + jg * C1P)
                            if r % 2 == 0:
                                nc.scalar.activation(dst, src, AF.Relu)
                            else:
                                nc.vector.tensor_scalar_max(dst, src, 0.0)

            # conv2: 16 streams (r, c) = 8 slots x 2 v; one [128, 27S] psum
            # bank per slot; 4 concurrent slots per wave (all 16 PE tiles),
            # o-outer emission so consecutive MMs hit different row groups
            for w0 in range(0, 8, 4):
                slots = list(range(w0, w0 + 4))
                pss2 = {sl: pspool.tile([128, 27 * S], mybir.dt.float32,
                                        tag="ps", name=f"ps2_{sl}")
                        for sl in slots}
                for o in range(27):
                    dz, dy, dx = o // 9, (o // 3) % 3, o % 3
                    for v in range(2):
                        for sl in slots:
                            r = (sl // 4) * 2 + v
                            c = sl % 4
                            rhs = _ap(
                                c1[:],
                                [[FC1, 32], [49, 3 * S], [14, 3], [2, 3]],
                                off=32 * c * FC1 + r * S * C1J
                                    + dz * S * C1P + dy * 7 + dx)
                            nc.tensor.matmul(
                                pss2[sl][64 * v:64 * v + 64, :],
                                w2[32 * c:32 * c + 32, o * 64:(o + 1) * 64],
                                rhs,
                                start=(o == 0), stop=(o == 26),
                                tile_position=(32 * c, 64 * v),
                            )
                for i, sl in enumerate(slots):
                    if i % 2 == 0:
                        nc.scalar.activation(ccr[:, sl, :, :], pss2[sl][:],
                                             AF.Relu, bias=b2[:, 0:1])
                    else:
                        nc.vector.tensor_scalar(
                            ccr[:, sl, :, :], pss2[sl][:], b2[:, 0:1], 0.0,
                            op0=mybir.AluOpType.add, op1=mybir.AluOpType.max)
            # fc1: contract (co64, pos27); N = (slot8, S) = 8S cols
            f1 = fpool.tile([128, 2 * 2 * 8 * S], mybir.dt.float32)
            f1r = f1[:].rearrange("p (h v n) -> p h v n", h=2, v=2)
            for h in range(2):
                psf = {v: pspool.tile([128, 8 * S], mybir.dt.float32,
                                      tag="ps", name=f"psf_{h}_{v}")
                       for v in range(2)}
                for pos in range(27):
                    for v in range(2):
                        nc.tensor.matmul(
                            psf[v][:],
                            wf1[64 * v:64 * v + 64,
                                pos * 256 + h * 128:pos * 256 + (h + 1) * 128],
                            ccr[64 * v:64 * v + 64, :, :, pos],
                            start=(pos == 0), stop=(pos == 26),
                            tile_position=(64 * v, 0),
                        )
                for v in range(2):
                    nc.scalar.activation(f1r[:, h, v, :], psf[v][:],
                                         AF.Relu, bias=bf1[:, h:h + 1])

            # fc2 (fp32) + tanh
            out_sb = fpool.tile([1, 2 * 8 * S], mybir.dt.float32)
            psf2 = pspool.tile([1, 2 * 8 * S], mybir.dt.float32, tag="ps")
            for h in range(2):
                nc.tensor.matmul(
                    psf2[:],
                    wf2[:, h:h + 1],
                    f1r[:, h, :, :],
                    start=(h == 0), stop=(h == 1),
                    tile_position=(0, 0),
                )
            nc.scalar.activation(out_sb[:], psf2[:], AF.Tanh, bias=bf2[0:1, 0:1])
            nc.sync.dma_start(o_d[:], out_sb[:])

    nc.compile()
    _cache[key] = nc
    return nc


def _bbox(mask):
    zs = np.flatnonzero(mask.any(axis=(1, 2)))
    ys = np.flatnonzero(mask.any(axis=(0, 2)))
    xs = np.flatnonzero(mask.any(axis=(0, 1)))
    return (int(xs[0]), int(ys[0]), int(zs[0]),
            int(xs[-1]), int(ys[-1]), int(zs[-1]))


def _extract(vol, bbox):
    x0, y0, z0, x1, y1, z1 = bbox
    t = vol[0, 0, z0:z1, y0:y1, x0:x1]
    pads = []
    for d in t.shape:
        rr = d % PATCH
        p = (PATCH - rr) % PATCH
        pads.append((p // 2, p - p // 2))
    t = np.pad(t, pads)
    D, H, W = t.shape
    nD, nH, nW = D // PATCH, H // PATCH, W // PATCH
    p = t.reshape(nD, PATCH, nH, PATCH, nW, PATCH)
    return p.transpose(0, 2, 4, 1, 3, 5).reshape(-1, PATCH, PATCH, PATCH)


def kernel(source, target, conv1_w, conv1_b, conv2_w, conv2_b,
           fc1_w, fc1_b, fc2_w, fc2_b):
    source = np.asarray(source, np.float32)
    target = np.asarray(target, np.float32)
    conv1_w = np.asarray(conv1_w, np.float32)
    conv1_b = np.asarray(conv1_b, np.float32)
    conv2_w = np.asarray(conv2_w, np.float32)
    conv2_b = np.asarray(conv2_b, np.float32)
    fc1_w = np.asarray(fc1_w, np.float32)
    fc1_b = np.asarray(fc1_b, np.float32)
    fc2_w = np.asarray(fc2_w, np.float32)
    fc2_b = np.asarray(fc2_b, np.float32)

    bbox = _bbox(target[0, 0] > 0)
    fixed = _extract(target, bbox)
    moving = _extract(source, bbox)
    Np = fixed.shape[0]
    keep = ((fixed == 0).reshape(Np, -1).mean(axis=1) <= THRESH).astype(np.float32)

    SLOTS = NCORES * NR * NCG  # 128 streams
    S = -(-Np // SLOTS)
    Jb = _pick_jb(S)
    NB = S // Jb
    Npad = SLOTS * S

    nc = _build(S, Jb)

    # --- patch data: partial im2col [n, (ci,dz,dy)=18, (oz7,oy7,x17)] ---
    P2 = np.zeros((Npad, 2, PATCH, PATCH, PATCH), np.float32)
    P2[:Np, 0] = fixed
    P2[:Np, 1] = moving
    s0, s1, s2, s3, s4 = P2.strides
    cols = np.lib.stride_tricks.as_strided(
        P2, (Npad, 2, 3, 3, 7, 7, 15),
        (s0, s1, s2, s3, 2 * s2, 2 * s3, s4))
    # slot order (core, r, c, b, j); device layout [core][b][r][32][c][j][735]
    colsr = cols.reshape(NCORES, NR, NCG, NB, Jb, 18, PF)
    X = np.zeros((NCORES, NB, NR, 32, NCG, Jb, PF), NPDT8)
    X[:, :, :, :18] = colsr.transpose(0, 3, 1, 5, 2, 4, 6)
    X[:, :, :, 18] = np.float32(1.0)

    # --- weights ---
    w1t = conv1_w.transpose(1, 2, 3, 4, 0).reshape(18, 3, 32)  # (ci,dz,dy),dx,co
    W1 = np.zeros((128, 3, 32), np.float32)
    for r in range(NR):
        W1[32 * r:32 * r + 18] = w1t
        W1[32 * r + 18, 0] = conv1_b  # bias row pairs with the ones data row
    W1 = W1.reshape(128, 96).astype(NPDT8)

    w2t = conv2_w.transpose(1, 2, 3, 4, 0).reshape(32, 27, 64)  # ci,(dzdydx),co
    W2 = np.zeros((128, 27 * 64), np.float32)
    for c in range(NCG):
        W2[32 * c:32 * c + 32] = w2t.reshape(32, -1)
    W2 = W2.astype(NPDT)

    wf1t = fc1_w.reshape(256, 64, 27).transpose(1, 2, 0)  # co, pos, oc
    WF1 = np.zeros((128, 27 * 256), np.float32)
    for v in range(2):
        WF1[64 * v:64 * v + 64] = wf1t.reshape(64, -1)
    WF1 = WF1.astype(NPDT)

    WF2 = fc2_w.reshape(2, 128).T.copy().astype(np.float32)      # [128, 2]
    B2 = np.tile(conv2_b, 2).reshape(128, 1).astype(np.float32)
    BF1 = fc1_b.reshape(2, 128).T.copy().astype(np.float32)
    BF2 = fc2_b.reshape(1, 1).astype(np.float32)

    FX = NCG * Jb * PF
    in_maps = []
    for core in range(NCORES):
        in_maps.append({
            "x": np.ascontiguousarray(X[core]).reshape(NB, 128, FX),
            "w1": W1, "w2": W2, "wf1": WF1, "wf2": WF2,
            "b2": B2, "bf1": BF1, "bf2": BF2,
        })

    res = bass_utils.run_bass_kernel_spmd(nc, in_maps, core_ids=list(range(NCORES)))
    global _last_results
    _last_results = res

    # --- gather: out col = v*(8S) + slot*S + j ; slot=(r//2)*4+c, v=r%2 ---
    y = np.zeros(Npad, np.float32)
    o = np.stack([res.results[core]["o"][0] for core in range(NCORES)])
    ov = o.reshape(NCORES, 2, 8, S)                    # core, v, slot, j
    for v in range(2):
        for slot in range(8):
            r = (slot // 4) * 2 + v
            c = slot % 4
            base = (r * NCG + c) * S
            for core in range(NCORES):
                y[core * NR * NCG * S + base:
                  core * NR * NCG * S + base + S] = ov[core, v, slot]

    out = np.sum(y[:Np] * keep) / np.sum(keep)
    return np.float32(out)



# revision 11
# speedup vs baseline: 1.4275x; 1.0504x over previous
"""Trainium2 Bass kernel for nn_DMMRLoss — matmul-count-minimized design.

Per core: 8 streams = (r'2 x c4), S=36 patches/stream, 288 patches.
  conv1: FULL im2col (x,y,z gathered; K=54 rows, fp8) -> ONE matmul per
    (patch, stream-tile): 288 MMs of N=343, no accumulation, no psum chains.
  evac: relu+bias+dz-gather PSUM->SBUF bf16 (c1 layout [r'][dz][j][oz',y,x]
    so conv2 fuses (j,oz') into one AP dim), ACT/DVE alternating.
  conv2: 432 MMs of N=486 (27 offsets x 8 streams x 2 j-halves), single
    psum pass, all 8 banks resident.
  fc1: 108 MMs of N=144 (27 pos x 2 oc-halves x 2 v), [64,128] stationaries.
  fc2: 2 MMs; host applies fc2 bias + tanh + weighted mean.

Rationale: on this bass->walrus toolchain every matmul costs ~50ns of
serialized LDWEIGHTS+dispatch+semaphore regardless of N (measured), so
total matmuls (288+432+108+2) is the main driver.
"""
import sys

sys.path.insert(0, '/opt/trn_rl_repo')

import numpy as np
import ml_dtypes

import concourse.bacc as bacc
import concourse.mybir as mybir
import concourse.tile as tile
from concourse import bass_utils
from concourse.ap import AP


PATCH = 17
THRESH = 0.5
NCORES = 8
NRP = 2            # r' row-halves (stream dim 1)
NCG = 4            # col groups (stream dim 2)
S = 36             # patches per stream
NG = 6             # X DMA groups
JG = S // NG       # patches per DMA group (6)
AF = mybir.ActivationFunctionType

DT = mybir.dt.bfloat16
NPDT = ml_dtypes.bfloat16
DT8 = mybir.dt.float8e4
NPDT8 = ml_dtypes.float8_e4m3

KROWS = 54            # full im2col rows: 2ci * 3dz * 3dy * 3dx
C1P = 343             # conv1 out positions (7^3)
C1G = 441             # dz-gathered size: 3dz * 3oz' * 49
FXS = NCG * S * C1P   # x free size per partition-row: 49392
FC1 = NRP * 3 * S * 147  # c1 free: r' x dz x j x (oz',y,x): 31752
NOUT = 2 * 8 * 18     # 288 outputs per core (v2 x slot8 x j18)


def _ap(a, dims, off=0):
    return AP(tensor=a.tensor, offset=a.offset + off, ap=[list(d) for d in dims])


_cache = {}


def _build():
    if 'nc' in _cache:
        return _cache['nc']

    nc = bacc.Bacc("TRN2", target_bir_lowering=False, debug=False,
                   num_devices=NCORES)

    x_d = nc.dram_tensor("x", (NG, NRP, KROWS, NCG * JG * C1P), DT8,
                         kind="ExternalInput")
    w1_d = nc.dram_tensor("w1", (128, 128), DT8, kind="ExternalInput")
    w2_d = nc.dram_tensor("w2", (128, 27 * 128), DT, kind="ExternalInput")
    wf1_d = nc.dram_tensor("wf1", (128, 54 * 128), DT, kind="ExternalInput")
    wf2_d = nc.dram_tensor("wf2", (128, 2), DT, kind="ExternalInput")
    b1_d = nc.dram_tensor("b1", (128, 1), mybir.dt.float32, kind="ExternalInput")
    b2_d = nc.dram_tensor("b2", (128, 1), mybir.dt.float32, kind="ExternalInput")
    bf1_d = nc.dram_tensor("bf1", (128, 2), mybir.dt.float32, kind="ExternalInput")
    o_d = nc.dram_tensor("o", (1, NOUT), mybir.dt.float32, kind="ExternalOutput")

    with tile.TileContext(nc) as tc:
        with (
            tc.tile_pool(name="const", bufs=1) as cpool,
            tc.tile_pool(name="xin", bufs=1) as xpool,
            tc.tile_pool(name="c1", bufs=1) as c1pool,
            tc.tile_pool(name="cc", bufs=1) as ccpool,
            tc.tile_pool(name="fin", bufs=1) as fpool,
            tc.tile_pool(name="ps", bufs=2, space="PSUM") as pspool,
        ):
            w1 = cpool.tile([128, 128], DT8)
            dum = cpool.tile([64, 512], DT8)
            nc.vector.memset(dum[:], 0.0)
            w2 = cpool.tile([128, 27 * 128], DT)
            wf1 = cpool.tile([128, 54 * 128], DT)
            wf2 = cpool.tile([128, 2], DT)
            b1 = cpool.tile([128, 1], mybir.dt.float32)
            b2 = cpool.tile([128, 1], mybir.dt.float32)
            bf1 = cpool.tile([128, 2], mybir.dt.float32)

            x = xpool.tile([128, FXS], DT8)
            c1 = c1pool.tile([128, FC1], DT)
            cc = ccpool.tile([128, 27 * 8 * 18], DT)   # [pos27][slot8][j18]
            f1 = fpool.tile([128, 2 * NOUT], DT)
            out_sb = fpool.tile([1, NOUT], mybir.dt.float32)

            # --- DMAs ---
            nc.gpsimd.dma_start(w1[:], w1_d[:])
            nc.sync.dma_start(b1[:], b1_d[:])
            nc.sync.dma_start(b2[:], b2_d[:])
            nc.sync.dma_start(bf1[:], bf1_d[:])
            nc.sync.dma_start(wf2[:], wf2_d[:])
            for g in range(NG):
                for rp in range(NRP):
                    eng = nc.sync if rp == 0 else nc.gpsimd
                    dst = _ap(x[:], [[FXS, KROWS], [S * C1P, NCG], [1, JG * C1P]],
                              off=64 * rp * FXS + g * JG * C1P)
                    src = _ap(x_d[g, rp],
                              [[NCG * JG * C1P, KROWS], [JG * C1P, NCG],
                               [1, JG * C1P]])
                    eng.dma_start(dst, src)
            nc.gpsimd.dma_start(w2[:], w2_d[:])
            nc.gpsimd.dma_start(wf1[:], wf1_d[:])

            # --- PE warmup during X DMA wait ---
            warm = pspool.tile([128, 2048], mybir.dt.float32, tag="ps",
                               name="warm")
            for _ in range(16):
                nc.tensor.matmul(warm[0:32, 0:343], w1[0:54, 0:32],
                                 dum[0:54, 0:343], start=True, stop=True,
                                 tile_position=(0, 0))

            xr = x[:].rearrange("p (c j f) -> p c j f", c=NCG, j=S)

            # --- conv1: 2 patch-idx per psum tile; bank = jloc*2 + r' ---
            for grp in range(S // 2):
                pt = pspool.tile([128, 2048], mybir.dt.float32, tag="ps",
                                 name=f"p1_{grp}")
                for jl in range(2):
                    j = grp * 2 + jl
                    for rp in range(NRP):
                        for c in range(NCG):
                            nc.tensor.matmul(
                                pt[32 * c:32 * c + 32,
                                   (jl * 2 + rp) * 512:(jl * 2 + rp) * 512 + 343],
                                w1[64 * rp:64 * rp + KROWS, 32 * c:32 * c + 32],
                                xr[64 * rp:64 * rp + KROWS, c, j, :],
                                start=True, stop=True,
                                tile_position=(64 * rp, 32 * c))
                # evac: relu + bias + dz-gather -> c1[r'][dz][j][147]
                # one copy per (rp, dz): both j's, contiguous 294-elem dst
                # range (disjoint across copies -> engines run in parallel)
                for rp in range(NRP):
                    for dz in range(3):
                        src = _ap(pt[:],
                                  [[2048, 128], [1024, 2], [98, 3], [1, 49]],
                                  off=rp * 512 + dz * 49)
                        dst = _ap(c1[:], [[FC1, 128], [147, 2], [1, 147]],
                                  off=rp * (3 * S * 147) + dz * (S * 147)
                                      + grp * 2 * 147)
                        if rp == 0:
                            nc.scalar.activation(dst, src, AF.Relu,
                                                 bias=b1[:, 0:1])
                        else:
                            nc.vector.tensor_scalar(
                                dst, src, b1[:, 0:1], 0.0,
                                op0=mybir.AluOpType.add,
                                op1=mybir.AluOpType.max)

            # --- conv2: 27 o x 8 streams x 2 j-halves, N=486 ---
            # stream (r', c); psum: p2a = c 0,1 ; p2b = c 2,3 ;
            # bank (c%2)*2 + jh ; partitions 64r' + co
            p2a = pspool.tile([128, 2048], mybir.dt.float32, tag="ps",
                              name="p2a")
            p2b = pspool.tile([128, 2048], mybir.dt.float32, tag="ps",
                              name="p2b")
            for o in range(27):
                dz, dy, dx = o // 9, (o // 3) % 3, o % 3
                for c in range(NCG):
                    pt2 = p2a if c < 2 else p2b
                    for rp in range(NRP):
                        for jh in range(2):
                            rhs = _ap(
                                c1[:],
                                [[FC1, 32], [49, 54], [14, 3], [2, 3]],
                                off=32 * c * FC1 + rp * (3 * S * 147)
                                    + dz * (S * 147) + jh * 18 * 147
                                    + dy * 7 + dx)
                            bo = ((c % 2) * 2 + jh) * 512
                            nc.tensor.matmul(
                                pt2[64 * rp:64 * rp + 64, bo:bo + 486],
                                w2[32 * c:32 * c + 32,
                                   o * 128 + 64 * rp:o * 128 + 64 * rp + 64],
                                rhs, start=(o == 0), stop=(o == 26),
                                tile_position=(32 * c, 64 * rp))
            # cc evac: [j][pos] -> [pos][slot][j] ; slot = c*2 + jh
            for ci2 in range(2):
                pt2 = p2a if ci2 == 0 else p2b
                for i in range(4):
                    c = ci2 * 2 + i // 2
                    jh = i % 2
                    sl = c * 2 + jh
                    src = _ap(pt2[:], [[2048, 128], [1, 27], [27, 18]],
                              off=i * 512)
                    dst = _ap(cc[:], [[27 * 144, 128], [18, 27], [1, 18]],
                              off=sl * 486)
                    if i % 2 == 0:
                        nc.scalar.activation(dst, src, AF.Relu,
                                             bias=b2[:, 0:1])
                    else:
                        nc.vector.tensor_scalar(
                            dst, src, b2[:, 0:1], 0.0,
                            op0=mybir.AluOpType.add, op1=mybir.AluOpType.max)

            # --- fc1: 27 pos x 2 oc-halves; separate banks per v ---
            psf0 = pspool.tile([128, 2048], mybir.dt.float32, tag="ps",
                               name="psf0")
            psf1 = pspool.tile([128, 2048], mybir.dt.float32, tag="ps",
                               name="psf1")
            psf = (psf0, psf1)
            for pos in range(27):
                for h in range(2):
                    ch = pos * 2 + h
                    for v in range(2):
                        rhsf = _ap(cc[:], [[27 * 144, 64], [486, 8], [1, 18]],
                                   off=64 * v * (27 * 144) + pos * 18)
                        nc.tensor.matmul(
                            psf[h][0:128, v * 512:v * 512 + 144],
                            wf1[64 * v:64 * v + 64, ch * 128:(ch + 1) * 128],
                            rhsf,
                            start=(pos == 0), stop=(pos == 26),
                            tile_position=(64 * v, 0))
            srcf0 = _ap(psf0[:], [[2048, 128], [512, 2], [1, 144]])
            dstf0 = _ap(f1[:], [[2 * NOUT, 128], [144, 2], [1, 144]])
            nc.scalar.activation(dstf0, srcf0, AF.Relu, bias=bf1[:, 0:1])
            srcf1 = _ap(psf1[:], [[2048, 128], [512, 2], [1, 144]])
            dstf1 = _ap(f1[:], [[2 * NOUT, 128], [144, 2], [1, 144]], off=NOUT)
            nc.vector.tensor_scalar(dstf1, srcf1, bf1[:, 1:2], 0.0,
                                    op0=mybir.AluOpType.add,
                                    op1=mybir.AluOpType.max)

            # --- fc2 (host applies bias + tanh) ---
            psf2 = pspool.tile([128, 2048], mybir.dt.float32, tag="ps",
                               name="psf2")
            for h in range(2):
                nc.tensor.matmul(psf2[0:1, 0:NOUT], wf2[:, h:h + 1],
                                 f1[:, h * NOUT:(h + 1) * NOUT],
                                 start=(h == 0), stop=(h == 1),
                                 tile_position=(0, 0))
            nc.scalar.copy(out_sb[:], psf2[0:1, 0:NOUT])
            nc.sync.dma_start(o_d[:], out_sb[:])

    nc.compile()
    _cache['nc'] = nc
    return nc


def _bbox(mask):
    zs = np.flatnonzero(mask.any(axis=(1, 2)))
    ys = np.flatnonzero(mask.any(axis=(0, 2)))
    xs = np.flatnonzero(mask.any(axis=(0, 1)))
    return (int(xs[0]), int(ys[0]), int(zs[0]),
            int(xs[-1]), int(ys[-1]), int(zs[-1]))


def _extract(vol, bbox):
    x0, y0, z0, x1, y1, z1 = bbox
    t = vol[0, 0, z0:z1, y0:y1, x0:x1]
    pads = []
    for d in t.shape:
        rr = d % PATCH
        p = (PATCH - rr) % PATCH
        pads.append((p // 2, p - p // 2))
    t = np.pad(t, pads)
    D, H, W = t.shape
    nD, nH, nW = D // PATCH, H // PATCH, W // PATCH
    p = t.reshape(nD, PATCH, nH, PATCH, nW, PATCH)
    return p.transpose(0, 2, 4, 1, 3, 5).reshape(-1, PATCH, PATCH, PATCH)


def kernel(source, target, conv1_w, conv1_b, conv2_w, conv2_b,
           fc1_w, fc1_b, fc2_w, fc2_b):
    source = np.asarray(source, np.float32)
    target = np.asarray(target, np.float32)
    conv1_w = np.asarray(conv1_w, np.float32)
    conv1_b = np.asarray(conv1_b, np.float32)
    conv2_w = np.asarray(conv2_w, np.float32)
    conv2_b = np.asarray(conv2_b, np.float32)
    fc1_w = np.asarray(fc1_w, np.float32)
    fc1_b = np.asarray(fc1_b, np.float32)
    fc2_w = np.asarray(fc2_w, np.float32)
    fc2_b = np.asarray(fc2_b, np.float32)

    bbox = _bbox(target[0, 0] > 0)
    fixed = _extract(target, bbox)
    moving = _extract(source, bbox)
    Np = fixed.shape[0]
    keep = ((fixed == 0).reshape(Np, -1).mean(axis=1) <= THRESH).astype(np.float32)

    Npad = NCORES * NRP * NCG * S   # 2304
    assert Np <= Npad

    nc = _build()

    # --- X: FULL im2col [54 rows=(ci,dz,dy,dx)] x [343=(oz,oy,ox)] ---
    P2 = np.zeros((Npad, 2, PATCH, PATCH, PATCH), np.float32)
    P2[:Np, 0] = fixed
    P2[:Np, 1] = moving
    s0, s1, s2, s3, s4 = P2.strides
    cols = np.lib.stride_tricks.as_strided(
        P2, (Npad, 2, 3, 3, 3, 7, 7, 7),
        (s0, s1, s2, s3, s4, 2 * s2, 2 * s3, 2 * s4))
    # patch p = ((core*2 + r')*4 + c)*S + j  ; device [g][r'][row54][c][jj][343]
    colsr = cols.reshape(NCORES, NRP, NCG, NG, JG, KROWS, C1P)
    X8 = np.ascontiguousarray(
        colsr.transpose(0, 3, 1, 5, 2, 4, 6)).astype(NPDT8)

    # --- weights ---
    w1t = conv1_w.transpose(1, 2, 3, 4, 0).reshape(KROWS, 32)  # (ci,dz,dy,dx),co
    W1 = np.zeros((2, 64, 4, 32), np.float32)   # [r'][row64][c][co]
    W1[:, :KROWS] = w1t[None, :, None, :]
    W1 = W1.reshape(128, 128).astype(NPDT8)

    w2t = conv2_w.transpose(1, 2, 3, 4, 0).reshape(32, 27, 64)  # ci,o,co
    W2 = np.zeros((4, 32, 27, 2, 64), np.float32)  # [c][ci][o][v][co]
    W2[:] = w2t[None, :, :, None, :]
    W2 = W2.reshape(128, 27 * 128).astype(NPDT)

    wf1t = fc1_w.reshape(2, 128, 64, 27)           # [h][oc][co][pos]
    A = wf1t.transpose(2, 3, 0, 1).reshape(64, 54 * 128)
    WF1 = np.concatenate([A, A], axis=0).astype(NPDT)

    WF2 = fc2_w.reshape(2, 128).T.copy().astype(NPDT)
    B1 = np.tile(conv1_b, 4).reshape(128, 1).astype(np.float32)
    B2 = np.tile(conv2_b, 2).reshape(128, 1).astype(np.float32)
    BF1 = fc1_b.reshape(2, 128).T.copy().astype(np.float32)

    in_maps = []
    for core in range(NCORES):
        in_maps.append({
            "x": np.ascontiguousarray(X8[core]).reshape(NG, NRP, KROWS,
                                                        NCG * JG * C1P),
            "w1": W1, "w2": W2, "wf1": WF1, "wf2": WF2,
            "b1": B1, "b2": B2, "bf1": BF1,
        })

    res = bass_utils.run_bass_kernel_spmd(nc, in_maps,
                                          core_ids=list(range(NCORES)))
    global _last_results
    _last_results = res

    # --- gather: out col = v*144 + slot*18 + jj ; slot=c*2+jh ---
    y = np.zeros(Npad, np.float32)
    o = np.stack([res.results[core]["o"][0] for core in range(NCORES)])
    ov = o.reshape(NCORES, 2, 8, 18)               # core, v=r', slot, jj
    for v in range(2):
        for sl in range(8):
            c = sl // 2
            jh = sl % 2
            base = ((v * NCG) + c) * S + jh * 18
            for core in range(NCORES):
                y[core * NRP * NCG * S + base:
                  core * NRP * NCG * S + base + 18] = ov[core, v, sl]

    yt = np.tanh(y + fc2_b[0])
    out = np.sum(yt[:Np] * keep) / np.sum(keep)
    return np.float32(out)


# revision 12
# speedup vs baseline: 1.6788x; 1.1760x over previous
"""Trainium2 Bass kernel for nn_DMMRLoss — matmul-count-minimized design.

Per core: 8 streams = (r'2 x c4), S=36 patches/stream, 288 patches.
  conv1: FULL im2col (x,y,z gathered; K=54 rows, fp8) -> ONE matmul per
    (patch, stream-tile): 288 MMs of N=343, no accumulation, no psum chains.
  evac: relu+bias+dz-gather PSUM->SBUF bf16 (c1 layout [r'][dz][j][oz',y,x]
    so conv2 fuses (j,oz') into one AP dim), ACT/DVE alternating.
  conv2: 432 MMs of N=486 (27 offsets x 8 streams x 2 j-halves), single
    psum pass, all 8 banks resident.
  fc1: 108 MMs of N=144 (27 pos x 2 oc-halves x 2 v), [64,128] stationaries.
  fc2: 2 MMs; host applies fc2 bias + tanh + weighted mean.

Rationale: on this bass->walrus toolchain every matmul costs ~50ns of
serialized LDWEIGHTS+dispatch+semaphore regardless of N (measured), so
total matmuls (288+432+108+2) is the main driver.
"""
import sys

sys.path.insert(0, '/opt/trn_rl_repo')

import numpy as np
import ml_dtypes

import concourse.bacc as bacc
import concourse.mybir as mybir
import concourse.tile as tile
from concourse import bass_utils
from concourse.ap import AP


PATCH = 17
THRESH = 0.5
NCORES = 8
NRP = 2            # r' row-halves (stream dim 1)
NCG = 4            # col groups (stream dim 2)
S = 36             # patches per stream
NG = 6             # X DMA groups
JG = S // NG       # patches per DMA group (6)
AF = mybir.ActivationFunctionType

DT = mybir.dt.bfloat16
NPDT = ml_dtypes.bfloat16
DT8 = mybir.dt.float8e4
NPDT8 = ml_dtypes.float8_e4m3

KROWS = 54            # full im2col rows: 2ci * 3dz * 3dy * 3dx
C1P = 343             # conv1 out positions (7^3)
C1G = 441             # dz-gathered size: 3dz * 3oz' * 49
FXS = NCG * S * C1P   # x free size per partition-row: 49392
FC1 = NRP * 3 * S * 147  # c1 free: r' x dz x j x (oz',y,x): 31752
NOUT = 2 * 8 * 18     # 288 outputs per core (v2 x slot8 x j18)


def _ap(a, dims, off=0):
    return AP(tensor=a.tensor, offset=a.offset + off, ap=[list(d) for d in dims])


_cache = {}


def _build():
    if 'nc' in _cache:
        return _cache['nc']

    nc = bacc.Bacc("TRN2", target_bir_lowering=False, debug=False,
                   num_devices=NCORES)

    x_d = nc.dram_tensor("x", (NG, 128, NCG * JG * C1P), DT8,
                         kind="ExternalInput")
    w1_d = nc.dram_tensor("w1", (128, 128), DT8, kind="ExternalInput")
    w2_d = nc.dram_tensor("w2", (128, 27 * 128), DT, kind="ExternalInput")
    wf1_d = nc.dram_tensor("wf1", (128, 54 * 128), DT, kind="ExternalInput")
    wf2_d = nc.dram_tensor("wf2", (128, 2), DT, kind="ExternalInput")
    b1_d = nc.dram_tensor("b1", (128, 1), mybir.dt.float32, kind="ExternalInput")
    b2_d = nc.dram_tensor("b2", (128, 1), mybir.dt.float32, kind="ExternalInput")
    bf1_d = nc.dram_tensor("bf1", (128, 2), mybir.dt.float32, kind="ExternalInput")
    o_d = nc.dram_tensor("o", (1, NOUT), mybir.dt.float32, kind="ExternalOutput")

    with tile.TileContext(nc) as tc:
        with (
            tc.tile_pool(name="const", bufs=1) as cpool,
            tc.tile_pool(name="xin", bufs=1) as xpool,
            tc.tile_pool(name="c1", bufs=1) as c1pool,
            tc.tile_pool(name="cc", bufs=1) as ccpool,
            tc.tile_pool(name="fin", bufs=1) as fpool,
            tc.tile_pool(name="ps", bufs=2, space="PSUM") as pspool,
        ):
            w1 = cpool.tile([128, 128], DT8)
            dum = cpool.tile([64, 512], DT8)
            nc.vector.memset(dum[:], 0.0)
            w2 = cpool.tile([128, 27 * 128], DT)
            wf1 = cpool.tile([128, 54 * 128], DT)
            wf2 = cpool.tile([128, 2], DT)
            b1 = cpool.tile([128, 1], mybir.dt.float32)
            b2 = cpool.tile([128, 1], mybir.dt.float32)
            bf1 = cpool.tile([128, 2], mybir.dt.float32)

            x = xpool.tile([128, FXS], DT8)
            c1 = c1pool.tile([128, FC1], DT)
            cc = ccpool.tile([128, 27 * 8 * 18], DT)   # [pos27][slot8][j18]
            f1 = fpool.tile([128, 2 * NOUT], DT)
            out_sb = fpool.tile([1, NOUT], mybir.dt.float32)

            # --- DMAs ---
            nc.gpsimd.dma_start(w1[:], w1_d[:])
            nc.sync.dma_start(b1[:], b1_d[:])
            nc.sync.dma_start(b2[:], b2_d[:])
            nc.sync.dma_start(bf1[:], bf1_d[:])
            nc.sync.dma_start(wf2[:], wf2_d[:])
            for g in range(NG):
                eng = nc.sync if g % 2 == 0 else nc.gpsimd
                dstx = _ap(x[:], [[FXS, 128], [S * C1P, NCG], [1, JG * C1P]],
                           off=g * JG * C1P)
                srcx = _ap(x_d[g],
                           [[NCG * JG * C1P, 128], [JG * C1P, NCG],
                            [1, JG * C1P]])
                eng.dma_start(dstx, srcx)
            nc.gpsimd.dma_start(w2[:], w2_d[:])
            nc.gpsimd.dma_start(wf1[:], wf1_d[:])

            # --- PE warmup during X DMA wait ---
            warm = pspool.tile([128, 2048], mybir.dt.float32, tag="ps",
                               name="warm")
            for _ in range(16):
                nc.tensor.matmul(warm[0:32, 0:343], w1[0:54, 0:32],
                                 dum[0:54, 0:343], start=True, stop=True,
                                 tile_position=(0, 0))

            xr = x[:].rearrange("p (c j f) -> p c j f", c=NCG, j=S)

            # --- conv1: 2 patch-idx per psum tile; bank = jloc*2 + r' ---
            for grp in range(S // 2):
                pt = pspool.tile([128, 2048], mybir.dt.float32, tag="ps",
                                 name=f"p1_{grp}")
                for jl in range(2):
                    j = grp * 2 + jl
                    for rp in range(NRP):
                        for c in range(NCG):
                            nc.tensor.matmul(
                                pt[32 * c:32 * c + 32,
                                   (jl * 2 + rp) * 512:(jl * 2 + rp) * 512 + 343],
                                w1[64 * rp:64 * rp + KROWS, 32 * c:32 * c + 32],
                                xr[64 * rp:64 * rp + KROWS, c, j, :],
                                start=True, stop=True,
                                tile_position=(64 * rp, 32 * c))
                # evac: relu + bias + dz-gather -> c1[r'][dz][j][147]
                # one copy per (rp, dz): both j's, contiguous 294-elem dst
                # range (disjoint across copies -> engines run in parallel)
                for rp in range(NRP):
                    for dz in range(3):
                        src = _ap(pt[:],
                                  [[2048, 128], [1024, 2], [98, 3], [1, 49]],
                                  off=rp * 512 + dz * 49)
                        dst = _ap(c1[:], [[FC1, 128], [147, 2], [1, 147]],
                                  off=rp * (3 * S * 147) + dz * (S * 147)
                                      + grp * 2 * 147)
                        if rp == 0:
                            nc.scalar.activation(dst, src, AF.Relu,
                                                 bias=b1[:, 0:1])
                        else:
                            nc.vector.tensor_scalar(
                                dst, src, b1[:, 0:1], 0.0,
                                op0=mybir.AluOpType.add,
                                op1=mybir.AluOpType.max)

            # --- conv2: 27 o x 8 streams x 2 j-halves, N=486 ---
            # stream (r', c); psum: p2a = c 0,1 ; p2b = c 2,3 ;
            # bank (c%2)*2 + jh ; partitions 64r' + co
            p2a = pspool.tile([128, 2048], mybir.dt.float32, tag="ps",
                              name="p2a")
            p2b = pspool.tile([128, 2048], mybir.dt.float32, tag="ps",
                              name="p2b")
            for o in range(27):
                dz, dy, dx = o // 9, (o // 3) % 3, o % 3
                for c in range(NCG):
                    pt2 = p2a if c < 2 else p2b
                    for rp in range(NRP):
                        for jh in range(2):
                            rhs = _ap(
                                c1[:],
                                [[FC1, 32], [49, 54], [14, 3], [2, 3]],
                                off=32 * c * FC1 + rp * (3 * S * 147)
                                    + dz * (S * 147) + jh * 18 * 147
                                    + dy * 7 + dx)
                            bo = ((c % 2) * 2 + jh) * 512
                            nc.tensor.matmul(
                                pt2[64 * rp:64 * rp + 64, bo:bo + 486],
                                w2[32 * c:32 * c + 32,
                                   o * 128 + 64 * rp:o * 128 + 64 * rp + 64],
                                rhs, start=(o == 0), stop=(o == 26),
                                tile_position=(32 * c, 64 * rp))
            # cc evac: [j][pos] -> [pos][slot][j] ; slot = c*2 + jh
            for ci2 in range(2):
                pt2 = p2a if ci2 == 0 else p2b
                for i in range(4):
                    c = ci2 * 2 + i // 2
                    jh = i % 2
                    sl = c * 2 + jh
                    src = _ap(pt2[:], [[2048, 128], [1, 27], [27, 18]],
                              off=i * 512)
                    dst = _ap(cc[:], [[27 * 144, 128], [18, 27], [1, 18]],
                              off=sl * 486)
                    if i % 2 == 0:
                        nc.scalar.activation(dst, src, AF.Relu,
                                             bias=b2[:, 0:1])
                    else:
                        nc.vector.tensor_scalar(
                            dst, src, b2[:, 0:1], 0.0,
                            op0=mybir.AluOpType.add, op1=mybir.AluOpType.max)

            # --- fc1: 27 pos x 2 oc-halves; separate banks per v ---
            psf0 = pspool.tile([128, 2048], mybir.dt.float32, tag="ps",
                               name="psf0")
            psf1 = pspool.tile([128, 2048], mybir.dt.float32, tag="ps",
                               name="psf1")
            psf = (psf0, psf1)
            for pos in range(27):
                for h in range(2):
                    ch = pos * 2 + h
                    for v in range(2):
                        rhsf = _ap(cc[:], [[27 * 144, 64], [486, 8], [1, 18]],
                                   off=64 * v * (27 * 144) + pos * 18)
                        nc.tensor.matmul(
                            psf[h][0:128, v * 512:v * 512 + 144],
                            wf1[64 * v:64 * v + 64, ch * 128:(ch + 1) * 128],
                            rhsf,
                            start=(pos == 0), stop=(pos == 26),
                            tile_position=(64 * v, 0))
            srcf0 = _ap(psf0[:], [[2048, 128], [512, 2], [1, 144]])
            dstf0 = _ap(f1[:], [[2 * NOUT, 128], [144, 2], [1, 144]])
            nc.scalar.activation(dstf0, srcf0, AF.Relu, bias=bf1[:, 0:1])
            srcf1 = _ap(psf1[:], [[2048, 128], [512, 2], [1, 144]])
            dstf1 = _ap(f1[:], [[2 * NOUT, 128], [144, 2], [1, 144]], off=NOUT)
            nc.vector.tensor_scalar(dstf1, srcf1, bf1[:, 1:2], 0.0,
                                    op0=mybir.AluOpType.add,
                                    op1=mybir.AluOpType.max)

            # --- fc2 (host applies bias + tanh) ---
            psf2 = pspool.tile([128, 2048], mybir.dt.float32, tag="ps",
                               name="psf2")
            for h in range(2):
                nc.tensor.matmul(psf2[0:1, 0:NOUT], wf2[:, h:h + 1],
                                 f1[:, h * NOUT:(h + 1) * NOUT],
                                 start=(h == 0), stop=(h == 1),
                                 tile_position=(0, 0))
            nc.scalar.copy(out_sb[:], psf2[0:1, 0:NOUT])
            nc.sync.dma_start(o_d[:], out_sb[:])

    nc.compile()
    _cache['nc'] = nc
    return nc


def _bbox(mask):
    zs = np.flatnonzero(mask.any(axis=(1, 2)))
    ys = np.flatnonzero(mask.any(axis=(0, 2)))
    xs = np.flatnonzero(mask.any(axis=(0, 1)))
    return (int(xs[0]), int(ys[0]), int(zs[0]),
            int(xs[-1]), int(ys[-1]), int(zs[-1]))


def _extract(vol, bbox):
    x0, y0, z0, x1, y1, z1 = bbox
    t = vol[0, 0, z0:z1, y0:y1, x0:x1]
    pads = []
    for d in t.shape:
        rr = d % PATCH
        p = (PATCH - rr) % PATCH
        pads.append((p // 2, p - p // 2))
    t = np.pad(t, pads)
    D, H, W = t.shape
    nD, nH, nW = D // PATCH, H // PATCH, W // PATCH
    p = t.reshape(nD, PATCH, nH, PATCH, nW, PATCH)
    return p.transpose(0, 2, 4, 1, 3, 5).reshape(-1, PATCH, PATCH, PATCH)


def kernel(source, target, conv1_w, conv1_b, conv2_w, conv2_b,
           fc1_w, fc1_b, fc2_w, fc2_b):
    source = np.asarray(source, np.float32)
    target = np.asarray(target, np.float32)
    conv1_w = np.asarray(conv1_w, np.float32)
    conv1_b = np.asarray(conv1_b, np.float32)
    conv2_w = np.asarray(conv2_w, np.float32)
    conv2_b = np.asarray(conv2_b, np.float32)
    fc1_w = np.asarray(fc1_w, np.float32)
    fc1_b = np.asarray(fc1_b, np.float32)
    fc2_w = np.asarray(fc2_w, np.float32)
    fc2_b = np.asarray(fc2_b, np.float32)

    bbox = _bbox(target[0, 0] > 0)
    fixed = _extract(target, bbox)
    moving = _extract(source, bbox)
    Np = fixed.shape[0]
    keep = ((fixed == 0).reshape(Np, -1).mean(axis=1) <= THRESH).astype(np.float32)

    Npad = NCORES * NRP * NCG * S   # 2304
    assert Np <= Npad

    nc = _build()

    # --- X: FULL im2col [54 rows=(ci,dz,dy,dx)] x [343=(oz,oy,ox)] ---
    P2 = np.zeros((Npad, 2, PATCH, PATCH, PATCH), np.float32)
    P2[:Np, 0] = fixed
    P2[:Np, 1] = moving
    s0, s1, s2, s3, s4 = P2.strides
    cols = np.lib.stride_tricks.as_strided(
        P2, (Npad, 2, 3, 3, 3, 7, 7, 7),
        (s0, s1, s2, s3, s4, 2 * s2, 2 * s3, 2 * s4))
    # patch p = ((core*2 + r')*4 + c)*S + j ; device [g][128=(r',64row)][c][jj][343]
    colsr = cols.reshape(NCORES, NRP, NCG, NG, JG, KROWS, C1P)
    ct = colsr.transpose(0, 3, 1, 5, 2, 4, 6)  # [core][g][rp][row54][c][jj][343]
    X8 = np.zeros((NCORES, NG, NRP, 64, NCG, JG, C1P), NPDT8)
    X8[:, :, :, :KROWS] = ct.astype(NPDT8)

    # --- weights ---
    w1t = conv1_w.transpose(1, 2, 3, 4, 0).reshape(KROWS, 32)  # (ci,dz,dy,dx),co
    W1 = np.zeros((2, 64, 4, 32), np.float32)   # [r'][row64][c][co]
    W1[:, :KROWS] = w1t[None, :, None, :]
    W1 = W1.reshape(128, 128).astype(NPDT8)

    w2t = conv2_w.transpose(1, 2, 3, 4, 0).reshape(32, 27, 64)  # ci,o,co
    W2 = np.zeros((4, 32, 27, 2, 64), np.float32)  # [c][ci][o][v][co]
    W2[:] = w2t[None, :, :, None, :]
    W2 = W2.reshape(128, 27 * 128).astype(NPDT)

    wf1t = fc1_w.reshape(2, 128, 64, 27)           # [h][oc][co][pos]
    A = wf1t.transpose(2, 3, 0, 1).reshape(64, 54 * 128)
    WF1 = np.concatenate([A, A], axis=0).astype(NPDT)

    WF2 = fc2_w.reshape(2, 128).T.copy().astype(NPDT)
    B1 = np.tile(conv1_b, 4).reshape(128, 1).astype(np.float32)
    B2 = np.tile(conv2_b, 2).reshape(128, 1).astype(np.float32)
    BF1 = fc1_b.reshape(2, 128).T.copy().astype(np.float32)

    in_maps = []
    for core in range(NCORES):
        in_maps.append({
            "x": np.ascontiguousarray(X8[core]).reshape(NG, 128,
                                                        NCG * JG * C1P),
            "w1": W1, "w2": W2, "wf1": WF1, "wf2": WF2,
            "b1": B1, "b2": B2, "bf1": BF1,
        })

    res = bass_utils.run_bass_kernel_spmd(nc, in_maps,
                                          core_ids=list(range(NCORES)))
    global _last_results
    _last_results = res

    # --- gather: out col = v*144 + slot*18 + jj ; slot=c*2+jh ---
    y = np.zeros(Npad, np.float32)
    o = np.stack([res.results[core]["o"][0] for core in range(NCORES)])
    ov = o.reshape(NCORES, 2, 8, 18)               # core, v=r', slot, jj
    for v in range(2):
        for sl in range(8):
            c = sl // 2
            jh = sl % 2
            base = ((v * NCG) + c) * S + jh * 18
            for core in range(NCORES):
                y[core * NRP * NCG * S + base:
                  core * NRP * NCG * S + base + 18] = ov[core, v, sl]

    yt = np.tanh(y + fc2_b[0])
    out = np.sum(yt[:Np] * keep) / np.sum(keep)
    return np.float32(out)
